# revision 9
# baseline (speedup 1.0000x reference)
"""Trainium2 Bass kernel for MimickedSelfContactLoss (retrieval_knn).

Math reduction: the reference builds the full N x N vertex distance matrix but
only ever reads it at (contact, contact) index pairs, and the argmin feeds a
gather of the *same* distance matrix, so

    loss = mean_i tanh( min_{j : geomask[pc_i, pc_j]} ||v[pc_i] - v[pc_j]|| )

i.e. a C x C (1024 x 1024) masked-min pairwise-distance problem over the
contact subset.  (If a row has no allowed neighbor the reference would pick
column 0; with a ~0.5-dense random mask over 1024 columns that case has
probability ~2^-1024 and is ignored.)

Distribution: row-shard the C x C computation across 8 NeuronCores -- each
core owns 128 query contacts vs all 1024 contacts (the sharding_hint's
row-wise split, applied to the contact subset, with its geomask rows sharded
alongside).  Per core:

  PE   : squared distances via one K=5 fp32 matmul using the
         |q|^2 + |k|^2 - 2 q.k expansion ([-2q; q^2; 1]^T [k; 1; k^2])
  ACT  : penalty = BIG * (1 - mask)  (exact 0 for allowed pairs)
  DVE  : fused tensor_tensor_reduce  min_j(dist2 + penalty)  -> [128,1]
  DVE  : threshold at TAU to restore exact zeros lost to fp32 cancellation
         in the matmul expansion (residual < 1e-5 << TAU << 2.4e-4 = smallest
         genuine nonzero contact dist^2)
  ACT  : sqrt, tanh -> [128,1]

The 8 cores return their 128 tanh values; the host concatenates (the
"all-gather") and takes the mean.
"""

import numpy as np
import ml_dtypes

import concourse.bass as bass
import concourse.mybir as mybir
import concourse.tile as tile
from concourse import bacc
from concourse.bass_utils import run_bass_kernel_spmd

N = 6890
C = 1024
NCORES = 8
P = C // NCORES          # 128 query rows per core
NCH = 2                  # free-dim chunks (fp32 matmul moving operand <= 512)
CH = C // NCH
BIG = float(2 ** 20)     # exact in bf16/f32; >> max contact dist^2 (~70)
TAU = 3e-5               # separates fp32 cancellation residue from real dist^2
INIT = 1e30              # min-reduction init


def build_nc() -> bass.Bass:
    # bacc (not raw Bass): its compile() pass legalizes multi-wait sync into
    # what walrus codegen accepts.
    nc = bacc.Bacc("TRN2", target_bir_lowering=False, debug=False)
    dt = mybir.dt

    # aug packs [aq | ak]: cols 0:P are the query block (lhsT), cols P:P+C the
    # key block (rhs) -- one DMA, one semaphore, so the matmul's sync-wait
    # count stays within the S3_LW slot limit.
    aug = nc.dram_tensor("aug", [5, P + C], dt.float32, kind="ExternalInput").ap()
    mask = nc.dram_tensor("mask", [P, C], dt.bfloat16, kind="ExternalInput").ap()
    out = nc.dram_tensor("out", [P, 1], dt.float32, kind="ExternalOutput").ap()

    with tile.TileContext(nc) as tc:
        with (
            tc.tile_pool(name="inp", bufs=1) as inp,
            tc.tile_pool(name="scr", bufs=2) as scrp,
            tc.tile_pool(name="stat", bufs=6) as stat,
            tc.tile_pool(name="ps", bufs=2, space="PSUM") as psp,
        ):
            aug_s = inp.tile([5, P + C], dt.float32)
            nc.gpsimd.dma_start(aug_s[:], aug[:])
            mask_s = inp.tile([P, C], dt.bfloat16)
            nc.sync.dma_start(mask_s[:], mask[:])

            pen_s = inp.tile([P, C], dt.float32)
            mins = stat.tile([P, NCH], dt.float32, tag="mins")
            for ch in range(NCH):
                sl = bass.ts(ch, CH)
                # penalty chunk: BIG - BIG*mask  (0 where allowed)
                nc.scalar.activation(
                    pen_s[:, sl], mask_s[:, sl],
                    mybir.ActivationFunctionType.Copy,
                    bias=BIG, scale=-BIG,
                )
                ps = psp.tile([P, CH], dt.float32)
                nc.tensor.matmul(
                    ps[:], aug_s[:, 0:P], aug_s[:, P + ch * CH : P + (ch + 1) * CH],
                    start=True, stop=True,
                )
                # (TENSOR_TENSOR_REDUCE faults on this runtime, so two DVE ops)
                scr = scrp.tile([P, CH], dt.float32, tag="scr")
                nc.vector.tensor_tensor(
                    out=scr[:], in0=ps[:], in1=pen_s[:, sl], op=mybir.AluOpType.add
                )
                nc.vector.tensor_reduce(
                    mins[:, ch : ch + 1], scr[:],
                    axis=mybir.AxisListType.X, op=mybir.AluOpType.min,
                )

            m_cur = stat.tile([P, 1], dt.float32, tag="m")
            nc.vector.tensor_reduce(
                m_cur[:], mins[:], axis=mybir.AxisListType.X, op=mybir.AluOpType.min
            )
            thr = stat.tile([P, 1], dt.float32, tag="thr")
            nc.vector.tensor_scalar(
                out=thr[:], in0=m_cur[:], scalar1=TAU, scalar2=None,
                op0=mybir.AluOpType.is_ge,
            )
            m2 = stat.tile([P, 1], dt.float32, tag="m2")
            nc.vector.tensor_mul(m2[:], m_cur[:], thr[:])
            v = stat.tile([P, 1], dt.float32, tag="v")
            nc.scalar.sqrt(v[:], m2[:])
            th = stat.tile([P, 1], dt.float32, tag="th")
            nc.scalar.activation(th[:], v[:], mybir.ActivationFunctionType.Tanh)
            nc.sync.dma_start(out[:], th[:])
    nc.compile()
    return nc


def prepare_in_maps(presented_contact, vertices, geomask):
    pc = np.asarray(presented_contact).astype(np.int64)
    verts = np.asarray(vertices, dtype=np.float32).reshape(N, 3)
    gm = np.asarray(geomask)

    vc = verts[pc]                                    # [C, 3]
    q2 = (vc * vc).sum(axis=1, dtype=np.float32)      # [C]
    ones = np.ones((1, C), np.float32)
    ak = np.concatenate([vc.T, ones, q2[None, :]], axis=0).astype(np.float32)
    mg = gm[pc][:, pc]                                # [C, C] bool
    mask_bf16 = mg.astype(ml_dtypes.bfloat16)         # 1.0 allowed / 0.0 not

    in_maps = []
    for g in range(NCORES):
        sl = slice(g * P, (g + 1) * P)
        aq = np.concatenate(
            [(-2.0 * vc[sl].T), q2[None, sl], np.ones((1, P), np.float32)], axis=0
        ).astype(np.float32)
        aug = np.concatenate([aq, ak], axis=1).astype(np.float32)   # [5, P+C]
        in_maps.append({
            "aug": np.ascontiguousarray(aug),
            "mask": np.ascontiguousarray(mask_bf16[sl]),
        })
    return in_maps


def finish(results) -> np.ndarray:
    th = np.concatenate([results[g]["out"][:, 0] for g in range(NCORES)])
    return np.asarray(th.astype(np.float64).mean(), dtype=np.float32)


def kernel(presented_contact, vertices, geomask) -> np.ndarray:
    in_maps = prepare_in_maps(presented_contact, vertices, geomask)
    nc = build_nc()
    res = run_bass_kernel_spmd(nc, in_maps, list(range(NCORES)))
    return finish(res.results)
